# revision 1
# baseline (speedup 1.0000x reference)
"""Trainium2 Bass kernel for a dense transformer block (self-attn + cross-attn + MLP).

Sharding: 8 cores = (batch b in 0..3) x (sequence half h in 0..1). Each core
computes its 512 query tokens end-to-end with no collectives: K/V projections
are recomputed locally (self-attn K/V over the full 1024 tokens of its batch,
cross-attn K/V over the full context). Causality is enforced with a
host-supplied additive mask so the single SPMD program is correct on both
halves.

All matmuls run in bf16 with f32 PSUM accumulation. LayerNorm gain/bias, the
1/sqrt(d) attention scale, and the K/V biases are folded into host-side
preprocessed weights (K-bias drops out of softmax entirely; V-bias is folded
into the output-projection bias).

Attention uses transposed scores [t_k, t_q] so the softmax denominator comes
free from an appended ones-column in V (M=65 matmuls); heads are packed in
pairs onto PE row-groups 0-63 / 64-127 to recover full rate at K=64.
"""
import os
from contextlib import ExitStack
import numpy as np
import ml_dtypes

P = 128
C = 1024
T = 1024
TL = 512     # local tokens per core
H = 16
D = 64
F = 4096
NKC = C // P      # 8 feature chunks
NTC = T // P      # 8 token chunks (kv)
NLC = TL // P     # 4 local token chunks
NFC = F // P      # 32 mlp hidden chunks
EPS = 1e-5
NEG = -50.0

_COMPILED = None


def _build():
    from concourse import bacc, tile
    import concourse.mybir as mybir
    F32 = mybir.dt.float32
    BF16 = mybir.dt.bfloat16
    ADD = mybir.AluOpType.add
    MULT = mybir.AluOpType.mult
    AF = mybir.ActivationFunctionType

    nc = bacc.Bacc("TRN2", target_bir_lowering=False, debug=False, num_devices=8)

    def param(name, shape, dt):
        return nc.declare_dram_parameter(name, list(shape), dt, isOutput=False)

    xloc = param("xloc", [TL, C], F32)
    xb16 = param("xb16", [T, C], BF16)
    ctx16 = param("ctx16", [T, C], BF16)
    w_ext = {}
    for nm, shape in [("wq_s", (C, C)), ("wk_s", (C, C)), ("wv_s", (C, C)),
                      ("wo_s", (C, C)), ("wq_c", (C, C)), ("wk_c", (C, C)),
                      ("wv_c", (C, C)), ("wo_c", (C, C)), ("w1f", (C, F)),
                      ("w2f", (F, C))]:
        w_ext[nm] = param(nm, shape, BF16)
    bq_s = param("bq_s", [P, NKC], F32)
    bq_c = param("bq_c", [P, NKC], F32)
    b1c = param("b1c", [P, NFC], F32)
    bo_s = param("bo_s", [1, C], BF16)
    bo_c = param("bo_c", [1, C], BF16)
    b2c = param("b2c", [1, C], BF16)
    mask16 = param("mask16", [T, TL], BF16)
    out_ext = nc.declare_dram_parameter("out", [TL, C], F32, isOutput=True)

    dbg = os.environ.get("KDBG", "")
    dbg_ext = None
    if dbg:
        dbg_ext = nc.declare_dram_parameter("dbg", [P, NFC, 1024], F32, isOutput=True)

    with TileCtx(nc, tile) as (tc, es):
        cst = es.enter_context(tc.tile_pool(name="cst", bufs=1))
        xlp = es.enter_context(tc.tile_pool(name="xlp", bufs=1))
        stg = es.enter_context(tc.tile_pool(name="stg", bufs=1))
        lntm = es.enter_context(tc.tile_pool(name="lntm", bufs=2))
        kvsrc = es.enter_context(tc.tile_pool(name="kvsrc", bufs=1))
        lnq = es.enter_context(tc.tile_pool(name="lnq", bufs=1))
        kfp = es.enter_context(tc.tile_pool(name="kfp", bufs=1))
        vap = es.enter_context(tc.tile_pool(name="vap", bufs=1))
        qfp = es.enter_context(tc.tile_pool(name="qfp", bufs=1))
        yfp = es.enter_context(tc.tile_pool(name="yfp", bufs=1))
        mkp = es.enter_context(tc.tile_pool(name="mkp", bufs=1))
        hfp = es.enter_context(tc.tile_pool(name="hfp", bufs=1))
        wp = es.enter_context(tc.tile_pool(name="wp", bufs=3))
        pp = es.enter_context(tc.tile_pool(name="pp", bufs=3))
        smp = es.enter_context(tc.tile_pool(name="smp", bufs=4))
        xmp = es.enter_context(tc.tile_pool(name="xmp", bufs=2))
        rbp = es.enter_context(tc.tile_pool(name="rbp", bufs=2))
        pw = es.enter_context(tc.tile_pool(name="pw", bufs=4, space="PSUM"))

        # constants
        ones_r = cst.tile([1, P], BF16, tag="ones_r")
        nc.gpsimd.memset(ones_r[:], 1.0)
        eps_t = cst.tile([P, 1], F32, tag="eps_t")
        nc.gpsimd.memset(eps_t[:], EPS)
        bq_s_sb = cst.tile([P, NKC], F32, tag="bq_s_sb")
        nc.sync.dma_start(out=bq_s_sb[:], in_=bq_s.ap())
        bq_c_sb = cst.tile([P, NKC], F32, tag="bq_c_sb")
        nc.sync.dma_start(out=bq_c_sb[:], in_=bq_c.ap())
        b1_sb = cst.tile([P, NFC], F32, tag="b1_sb")
        nc.sync.dma_start(out=b1_sb[:], in_=b1c.ap())
        bo_s_sb = cst.tile([1, C], BF16, tag="bo_s_sb")
        nc.sync.dma_start(out=bo_s_sb[:], in_=bo_s.ap())
        bo_c_sb = cst.tile([1, C], BF16, tag="bo_c_sb")
        nc.sync.dma_start(out=bo_c_sb[:], in_=bo_c.ap())
        b2_sb = cst.tile([1, C], BF16, tag="b2_sb")
        nc.sync.dma_start(out=b2_sb[:], in_=b2c.ap())

        # persistent activations
        x_loc = xlp.tile([P, NLC, C], F32, tag="x_loc")
        xloc_r = xloc.ap().rearrange("(c p) f -> p c f", p=P)

        SUB = mybir.AluOpType.subtract

        def ln_chunk(src, dst):
            """dst(bf16) = (src - mean(src)) * rsqrt(var(src)+eps), free dim C.

            Single pass over src: sum and sum-of-squares via ACT accumulators,
            then var = E[x^2] - mu^2 and dst = x*rstd - mu*rstd.
            """
            s = smp.tile([P, 4], F32, tag="lnstats")
            nc.vector.tensor_reduce(out=s[:, 0:1], in_=src, axis=mybir.AxisListType.X,
                                    op=ADD)
            sc2 = xmp.tile([P, C], BF16, tag="xm")
            nc.scalar.activation(sc2[:], src, AF.Square, accum_out=s[:, 1:2])
            nc.vector.tensor_scalar_mul(s[:, 2:3], s[:, 0:1], 1.0 / C)   # mu
            nc.vector.tensor_scalar_mul(s[:, 0:1], s[:, 1:2], 1.0 / C)   # E[x^2]
            nc.vector.tensor_tensor(out=s[:, 1:2], in0=s[:, 2:3], in1=s[:, 2:3],
                                    op=MULT)                              # mu^2
            nc.vector.tensor_tensor(out=s[:, 1:2], in0=s[:, 0:1], in1=s[:, 1:2],
                                    op=SUB)                               # var
            nc.scalar.activation(s[:, 3:4], s[:, 1:2], AF.Sqrt, bias=eps_t[:])
            nc.vector.reciprocal(s[:, 0:1], s[:, 3:4])                    # rstd
            nc.vector.tensor_tensor(out=s[:, 1:2], in0=s[:, 2:3], in1=s[:, 0:1],
                                    op=MULT)                              # mu*rstd
            nc.vector.tensor_scalar(out=dst, in0=src, scalar1=s[:, 0:1],
                                    scalar2=s[:, 1:2], op0=MULT, op1=SUB)

        def transpose_chunk(dst_fm, tcx, src_ap):
            """src [128 tok, 1024 feat] -> dst_fm[:, :, tcx-block] in one XBAR DMA."""
            nc.sync.dma_start_transpose(dst_fm[:, :, P * tcx:P * (tcx + 1)], src_ap)

        def ln_transpose(src_chunks, n_chunks, dst_fm):
            for tcx in range(n_chunks):
                lt = lntm.tile([P, C], BF16, tag="lntm")
                ln_chunk(src_chunks[:, tcx, :], lt[:])
                transpose_chunk(dst_fm, tcx, lt[:])

        # ---- stage 1: layernorms + transposes ----
        ln1kv_fm = kvsrc.tile([P, NKC, T], BF16, tag="kvsrc")
        xb16_r = xb16.ap().rearrange("(c p) f -> p c f", p=P)
        for half in range(2):
            x_half = stg.tile([P, NLC, C], BF16, tag="stg")
            for tcx in range(NLC):
                nc.sync.dma_start(out=x_half[:, tcx, :],
                                  in_=xb16_r[:, NLC * half + tcx, :])
                lt = lntm.tile([P, C], BF16, tag="lntm")
                ln_chunk(x_half[:, tcx, :], lt[:])
                transpose_chunk(ln1kv_fm, NLC * half + tcx, lt[:])
        ln1q_fm = lnq.tile([P, NKC, TL], BF16, tag="lnq")
        for tcx in range(NLC):
            nc.sync.dma_start(out=x_loc[:, tcx, :], in_=xloc_r[:, tcx, :])
            lt = lntm.tile([P, C], BF16, tag="lntm")
            ln_chunk(x_loc[:, tcx, :], lt[:])
            transpose_chunk(ln1q_fm, tcx, lt[:])

        def load_w_piece(wext, nh):
            """load [128, 8, 512] bf16 weight piece (cols nh*512..)."""
            wt = wp.tile([P, NKC, 512], BF16, tag="wp")
            nc.sync.dma_start(
                out=wt[:],
                in_=wext.ap().rearrange("(c p) n -> p c n", p=P)[:, :,
                                                                 512 * nh:512 * (nh + 1)])
            return wt

        def proj_fm(wext, src_fm, n_tok, dst, bias_sb=None):
            """dst[:, m, :n_tok] (feature-major, bf16) = W.T @ src_fm (+ bias per feat)."""
            nth = n_tok // 512
            for mh in range(2):
                wt = load_w_piece(wext, mh)
                for m4 in range(4):
                    m = 4 * mh + m4
                    ps = pw.tile([P, 1024], F32, tag="pw")
                    for th in range(nth):
                        for kc in range(NKC):
                            nc.tensor.matmul(ps[:, 512 * th:512 * (th + 1)],
                                             wt[:, kc, P * m4:P * (m4 + 1)],
                                             src_fm[:, kc, 512 * th:512 * (th + 1)],
                                             start=(kc == 0), stop=(kc == NKC - 1))
                    d = dst[:, m, 0:n_tok]
                    if bias_sb is None:
                        nc.vector.tensor_copy(out=d, in_=ps[:, 0:n_tok])
                    else:
                        nc.vector.tensor_scalar_add(d, ps[:, 0:n_tok],
                                                    bias_sb[:, m:m + 1])

        def proj_v(wext, src_fm, v_aug):
            """v_aug[:, tc, head, :] = token-major V projection into head slots.

            Every head slot: v at cols 0:64, ones at col 64 (so the softmax
            denominator lands on psum partition 64).
            """
            nc.gpsimd.memset(v_aug[:, :, :, D:D + 1], 1.0)
            for fh in range(2):
                wt = load_w_piece(wext, fh)
                for tcx in range(0, NTC, 2):
                    ps = pw.tile([P, 1024], F32, tag="pw")
                    for half in range(2):
                        for kc in range(NKC):
                            nc.tensor.matmul(
                                ps[:, 512 * half:512 * (half + 1)],
                                src_fm[:, kc, P * (tcx + half):P * (tcx + half + 1)],
                                wt[:, kc, :],
                                start=(kc == 0), stop=(kc == NKC - 1))
                        pv = ps[:, 512 * half:512 * (half + 1)].rearrange(
                            "p (hh d) -> p hh d", d=D)
                        nc.vector.tensor_copy(
                            out=v_aug[:, tcx + half, 8 * fh:8 * (fh + 1), 0:D],
                            in_=pv[:])

        def attention(q_fm, k_fm, v_aug, mask_sb, y_fm):
            for pr in range(H // 2):
                ys = pw.tile([P, 1024], F32, tag="pw", name=f"py_{pr}")
                for kc in range(NTC):
                    # cols < cs are masked on every core (t_k chunk entirely in
                    # the future of q block) -> skip scores/exp there
                    cs = 0 if mask_sb is None else P * max(0, kc - 4)
                    nq = TL - cs
                    sp = pw.tile([P, 1024], F32, tag="pw", name=f"sp_{pr}_{kc}")
                    for hh in range(2):
                        base = D * hh
                        nc.tensor.matmul(sp[:, 512 * hh + cs:512 * (hh + 1)],
                                         k_fm[base:base + D, pr, P * kc:P * (kc + 1)],
                                         q_fm[base:base + D, pr, cs:TL],
                                         start=True, stop=True)
                    pt = pp.tile([P, 1024], BF16, tag="pp")
                    ptv = pt[:].rearrange("p (e j) -> p e j", e=2)
                    spv = sp[:].rearrange("p (e j) -> p e j", e=2)
                    if mask_sb is not None:
                        if cs > 0:
                            nc.gpsimd.memset(ptv[:, :, 0:cs], 0.0)
                        praw = lntm.tile([P, 1024], BF16, tag="praw")
                        prv = praw[:].rearrange("p (e j) -> p e j", e=2)
                        nc.scalar.activation(prv[:, :, cs:TL], spv[:, :, cs:TL],
                                             AF.Exp)
                        nc.vector.tensor_tensor(
                            out=ptv[:, :, cs:TL],
                            in0=prv[:, :, cs:TL],
                            in1=mask_sb[:, kc, None, cs:TL].to_broadcast(
                                [P, 2, nq]),
                            op=MULT)
                    else:
                        nc.scalar.activation(pt[:], sp[:], AF.Exp)
                    for hh in range(2):
                        nc.tensor.matmul(ys[0:D + 1, 512 * hh:512 * (hh + 1)],
                                         v_aug[:, kc, 2 * pr + hh, :],
                                         pt[:, 512 * hh:512 * (hh + 1)],
                                         start=(kc == 0), stop=(kc == NTC - 1))
                for hh in range(2):
                    ysl = ys[:, 512 * hh:512 * (hh + 1)]
                    rs = rbp.tile([P, 512], F32, tag="rs")
                    nc.vector.reciprocal(rs[D:D + 1, :], ysl[D:D + 1, :])
                    rs2 = rbp.tile([1, 512], F32, tag="rs2")
                    nc.sync.dma_start(out=rs2[0:1, :], in_=rs[D:D + 1, :])
                    rb = rbp.tile([P, 512], F32, tag="rb")
                    nc.gpsimd.partition_broadcast(rb[:], rs2[0:1, :])
                    if hh == 0:
                        nc.vector.tensor_tensor(out=y_fm[0:D, pr, :],
                                                in0=ysl[0:D, :], in1=rb[0:D, :],
                                                op=MULT)
                    else:
                        yt = rbp.tile([P, 512], BF16, tag="yt")
                        nc.vector.tensor_tensor(out=yt[0:D, :], in0=ysl[0:D, :],
                                                in1=rb[0:D, :], op=MULT)
                        nc.sync.dma_start(out=y_fm[D:2 * D, pr, :], in_=yt[0:D, :])

        def out_proj(wext, y_fm, bias_row, dst_x):
            w0 = load_w_piece(wext, 0)
            w1 = load_w_piece(wext, 1)
            for tq in range(NLC):
                po = pw.tile([P, 1024], F32, tag="pw")
                for nh, wt in ((0, w0), (1, w1)):
                    ph = po[:, 512 * nh:512 * (nh + 1)]
                    for fc in range(NKC):
                        nc.tensor.matmul(ph,
                                         y_fm[:, fc, P * tq:P * (tq + 1)],
                                         wt[:, fc, :],
                                         start=(fc == 0), stop=False)
                    nc.tensor.matmul(ph, ones_r[0:1, :],
                                     bias_row[0:1, 512 * nh:512 * (nh + 1)],
                                     start=False, stop=True)
                nc.vector.tensor_tensor(out=dst_x[:, tq, :], in0=po[:],
                                        in1=dst_x[:, tq, :], op=ADD)

        # ---- stage 2: self attention ----
        k_fm = kfp.tile([P, NKC, T], BF16, tag="kfm")
        proj_fm(w_ext["wk_s"], ln1kv_fm, T, k_fm)
        v_aug = vap.tile([P, NTC, H, D + 1], BF16, tag="vaug")
        proj_v(w_ext["wv_s"], ln1kv_fm, v_aug)
        q_fm = qfp.tile([P, NKC, TL], BF16, tag="qfm")
        proj_fm(w_ext["wq_s"], ln1q_fm, TL, q_fm, bias_sb=bq_s_sb)
        mask_sb = mkp.tile([P, NTC, TL], BF16, tag="mask")
        nc.sync.dma_start(out=mask_sb[:],
                          in_=mask16.ap().rearrange("(c p) j -> p c j", p=P))
        y_fm = yfp.tile([P, NKC, TL], BF16, tag="yfm")
        attention(q_fm, k_fm, v_aug, mask_sb, y_fm)
        out_proj(w_ext["wo_s"], y_fm, bo_s_sb, x_loc)

        # ---- stage 3: cross attention ----
        ln2q_fm = lnq.tile([P, NKC, TL], BF16, tag="lnq")
        ln_transpose(x_loc, NLC, ln2q_fm)
        ctx_fm = kvsrc.tile([P, NKC, T], BF16, tag="kvsrc")
        ctx_r = ctx16.ap().rearrange("(c p) f -> p c f", p=P)
        for half in range(2):
            ctx_half = stg.tile([P, NLC, C], BF16, tag="stg")
            nc.sync.dma_start(out=ctx_half[:], in_=ctx_r[:, NLC * half:NLC * (half + 1), :])
            for tcx in range(NLC):
                transpose_chunk(ctx_fm, NLC * half + tcx, ctx_half[:, tcx, :])
        k_fm2 = kfp.tile([P, NKC, T], BF16, tag="kfm")
        proj_fm(w_ext["wk_c"], ctx_fm, T, k_fm2)
        v_aug2 = vap.tile([P, NTC, H, D + 1], BF16, tag="vaug")
        proj_v(w_ext["wv_c"], ctx_fm, v_aug2)
        q_fm2 = qfp.tile([P, NKC, TL], BF16, tag="qfm")
        proj_fm(w_ext["wq_c"], ln2q_fm, TL, q_fm2, bias_sb=bq_c_sb)
        y_fm2 = yfp.tile([P, NKC, TL], BF16, tag="yfm")
        attention(q_fm2, k_fm2, v_aug2, None, y_fm2)
        out_proj(w_ext["wo_c"], y_fm2, bo_c_sb, x_loc)

        # ---- stage 4: mlp ----
        ln2b_fm = lnq.tile([P, NKC, TL], BF16, tag="lnq")
        ln_transpose(x_loc, NLC, ln2b_fm)
        h_fm = hfp.tile([P, NFC, TL], BF16, tag="hfm")
        for piece in range(8):
            wt = load_w_piece(w_ext["w1f"], piece)
            for m4 in range(4):
                mc = 4 * piece + m4
                ps = pw.tile([P, 1024], F32, tag="pw")
                for kc in range(NKC):
                    nc.tensor.matmul(ps[:, 0:512], wt[:, kc, P * m4:P * (m4 + 1)],
                                     ln2b_fm[:, kc, :],
                                     start=(kc == 0), stop=(kc == NKC - 1))
                nc.scalar.activation(h_fm[:, mc, :], ps[:, 0:512], AF.Gelu,
                                     bias=b1_sb[:, mc:mc + 1])
        w2r = w_ext["w2f"].ap().rearrange("(c p) n -> p c n", p=P)
        pos = [pw.tile([P, 1024], F32, tag="pw", name=f"po_mlp_{i}")
               for i in range(NLC)]
        for kg in range(4):
            for nh in range(2):
                wt = wp.tile([P, NKC, 512], BF16, tag="wp")
                nc.sync.dma_start(out=wt[:],
                                  in_=w2r[:, NKC * kg:NKC * (kg + 1),
                                          512 * nh:512 * (nh + 1)])
                for tq in range(NLC):
                    for kc in range(NKC):
                        hc = NKC * kg + kc
                        nc.tensor.matmul(pos[tq][:, 512 * nh:512 * (nh + 1)],
                                         h_fm[:, hc, P * tq:P * (tq + 1)],
                                         wt[:, kc, :],
                                         start=(kg == 0 and kc == 0), stop=False)
        out_r = out_ext.ap().rearrange("(c p) f -> p c f", p=P)
        for tq in range(NLC):
            for nh in range(2):
                nc.tensor.matmul(pos[tq][:, 512 * nh:512 * (nh + 1)], ones_r[0:1, :],
                                 b2_sb[0:1, 512 * nh:512 * (nh + 1)],
                                 start=False, stop=True)
            nc.vector.tensor_tensor(out=x_loc[:, tq, :], in0=pos[tq][:],
                                    in1=x_loc[:, tq, :], op=ADD)
            nc.sync.dma_start(out=out_r[:, tq, :], in_=x_loc[:, tq, :])

        # ---- output (streamed per chunk above) ----

        if dbg_ext is not None:
            tap = {"ln1kv": ln1kv_fm, "kfm": k_fm, "qfm": q_fm, "yfm": y_fm,
                   "ln2q": ln2q_fm, "ctxfm": ctx_fm, "kfm2": k_fm2, "qfm2": q_fm2,
                   "yfm2": y_fm2, "ln2b": ln2b_fm, "hfm": h_fm}[dbg]
            sh = tap.shape
            nc.gpsimd.dma_start(out=dbg_ext.ap()[:, 0:sh[1], 0:sh[2]], in_=tap[:])

    nc.compile()
    return nc


class TileCtx:
    """TileContext plus an ExitStack for pools that closes before the context."""

    def __init__(self, nc, tile_mod):
        self._tc = tile_mod.TileContext(nc)
        self._es = ExitStack()

    def __enter__(self):
        tc = self._tc.__enter__()
        self._es.__enter__()
        return tc, self._es

    def __exit__(self, *exc):
        self._es.__exit__(*exc)
        return self._tc.__exit__(*exc)


def _get_compiled():
    global _COMPILED
    if _COMPILED is None:
        _COMPILED = _build()
    return _COMPILED


def _prep_inputs(x, context, ln1_g, ln1_b, ln2_g, ln2_b,
                 sa_wq, sa_bq, sa_wk, sa_bk, sa_wv, sa_bv, sa_wo, sa_bo,
                 ca_wq, ca_bq, ca_wk, ca_bk, ca_wv, ca_bv, ca_wo, ca_bo,
                 mlp_w1, mlp_b1, mlp_w2, mlp_b2):
    bf = ml_dtypes.bfloat16
    f32 = np.float32

    def fold(g, w, scale=1.0):
        return ((g[:, None] * w) * scale).astype(bf)

    shared = {
        "wq_s": fold(ln1_g, sa_wq, 0.125),
        "wk_s": fold(ln1_g, sa_wk),
        "wv_s": fold(ln1_g, sa_wv),
        "wo_s": sa_wo.astype(bf),
        "wq_c": fold(ln2_g, ca_wq, 0.125),
        "wk_c": ca_wk.astype(bf),
        "wv_c": ca_wv.astype(bf),
        "wo_c": ca_wo.astype(bf),
        "w1f": fold(ln2_g, mlp_w1),
        "w2f": mlp_w2.astype(bf),
        "bq_s": (((ln1_b @ sa_wq + sa_bq) * 0.125).astype(f32)
                 .reshape(NKC, P).T.copy()),
        "bq_c": (((ln2_b @ ca_wq + ca_bq) * 0.125).astype(f32)
                 .reshape(NKC, P).T.copy()),
        "b1c": (ln2_b @ mlp_w1 + mlp_b1).astype(f32).reshape(NFC, P).T.copy(),
        "bo_s": (sa_bo + (ln1_b @ sa_wv + sa_bv) @ sa_wo).reshape(1, C).astype(bf),
        "bo_c": (ca_bo + ca_bv @ ca_wo).reshape(1, C).astype(bf),
        "b2c": np.asarray(mlp_b2).reshape(1, C).astype(bf),
    }
    masks = []
    for h in range(2):
        off = TL * h
        tk = np.arange(T)[:, None]
        j = np.arange(TL)[None, :]
        masks.append(np.where(tk <= off + j, 1.0, 0.0).astype(bf))
    in_maps = []
    for core in range(8):
        b, h = core // 2, core % 2
        m = dict(shared)
        m["xloc"] = np.ascontiguousarray(x[b, TL * h:TL * (h + 1)]).astype(f32)
        m["xb16"] = x[b].astype(bf)
        m["ctx16"] = context[b].astype(bf)
        m["mask16"] = masks[h]
        in_maps.append(m)
    return in_maps


def kernel(**inputs):
    from concourse.bass_utils import run_bass_kernel_spmd
    nc = _get_compiled()
    inputs = {k: np.asarray(v) for k, v in inputs.items()}
    in_maps = _prep_inputs(**inputs)
    res = run_bass_kernel_spmd(nc, in_maps, core_ids=list(range(8)))
    out = np.empty((4, T, C), np.float32)
    for core in range(8):
        b, h = core // 2, core % 2
        out[b, TL * h:TL * (h + 1)] = res.results[core]["out"]
    return out

